# revision 8
# baseline (speedup 1.0000x reference)
"""Fused linear + cross-entropy loss (BaseChunkLoss) on 8 trn2 NeuronCores.

Strategy: 2-way token x 4-way vocab sharding (grid (i, j), core c = i*4 + j).
  - Tokens (N=8192) split in 2 halves of T=4096; vocab (V=32000) split in 4
    quarters of VC=8000. Each core computes the partial exp-sum of its token
    half over its vocab quarter; the host adds the 4 partials per token and
    takes log (the cross-device logsumexp of the sharding hint, done on the
    scalar-sized partials host-side, standing in for the wrapper's
    all_reduce).
  - Per-core HBM traffic is ~26 MB (fp8 weights quarter + fp8 hidden half +
    f32 rowdot operands), far below the tensor-engine time, so the kernel
    runs at the PE roofline: fp8e4 DoubleRow matmuls (K=256 per pass,
    0.5 cycles/column) = ~427 us of PE work per core.
  - Quantization to fp8 (weights pre-scaled by 64 for e4m3 range) happens on
    host; the device descales during the bias add, exactly matching the
    numerics of the on-device-converted fp8 baseline (~7.6e-5 rel err).
  - The target logit is computed exactly in f32: host gathers W[labels], the
    device does the per-token rowdot on the (otherwise idle) Pool/GpSimd
    engine with an accumulating scalar_tensor_tensor. Bias is added on host.

Device pipeline per (vocab-group g, token-block m, half): 16 DoubleRow
matmuls accumulate psum [128 tok x 2 banks x 500 vocab]; DVE does
(psum/64 + bias) in place; ACT computes exp with a fused row-sum
accumulator into s_cols. Per-bank-pair psum tiles (4 in flight) keep the
drain chain off the PE critical path.
"""
import numpy as np
import ml_dtypes
from contextlib import ExitStack

from concourse import bacc, mybir, tile
from concourse.bass_utils import run_bass_kernel_spmd

F32 = mybir.dt.float32
FP8 = mybir.dt.float8e4
Alu = mybir.AluOpType
Act = mybir.ActivationFunctionType

N_CORES = 8
N_TOK = 8192
D = 2048
V = 32000
P = 128

TOK_SPLIT = 2
VOC_SPLIT = 4
T = N_TOK // TOK_SPLIT        # 4096 tokens per core
VC = V // VOC_SPLIT           # 8000 vocab per core
KP = D // 256                 # 8 DoubleRow contraction passes of K=256
GV = 2000                     # vocab columns per W group (4 psum banks)
NG = VC // GV                 # 4 groups per core
MB = T // P                   # 32 token blocks per core
MBQ = MB // VOC_SPLIT         # 8 rowdot token blocks per core (1024 tokens)
BANK = 500                    # columns per psum bank

W_SCALE = 64.0                # fp8 weight pre-scale (e4m3 range)
FP8NP = ml_dtypes.float8_e4m3


def _build():
    nc = bacc.Bacc("TRN2", target_bir_lowering=False, debug=False)
    h_d = nc.declare_dram_parameter("h", [P, KP, 2, T], FP8, isOutput=False)
    W_d = nc.declare_dram_parameter("W", [P, KP, 2, VC], FP8, isOutput=False)
    bias_d = nc.declare_dram_parameter("bias", [VC], F32, isOutput=False)
    hn_d = nc.declare_dram_parameter("hn", [MBQ * P, D], F32, isOutput=False)
    wg_d = nc.declare_dram_parameter("wg", [MBQ * P, D], F32, isOutput=False)
    s_out = nc.declare_dram_parameter("s_out", [P, MB], F32, isOutput=True)
    t_out = nc.declare_dram_parameter("t_out", [P, MBQ], F32, isOutput=True)

    # h streams in token-chunks: each chunk carries ALL contraction passes
    # for 4 m-blocks, so the pipeline reaches full rate after ~1 MB of h
    HC = 512                                           # tokens per h chunk
    h_r = h_d[:].rearrange("p kp j (c t) -> c p kp j t", t=HC)
    W_r = W_d[:]                                       # [128, KP, 2, VC]

    with tile.TileContext(nc) as tc, ExitStack() as ctx:
        hpool = ctx.enter_context(tc.tile_pool(name="hT", bufs=1))
        wpool = ctx.enter_context(tc.tile_pool(name="w", bufs=2))
        bpool = ctx.enter_context(tc.tile_pool(name="bias", bufs=2))
        pspool = ctx.enter_context(tc.tile_pool(name="ps", bufs=4, space="PSUM"))
        epool = ctx.enter_context(tc.tile_pool(name="ejunk", bufs=2))
        hgpool = ctx.enter_context(tc.tile_pool(name="hg", bufs=2))
        wgpool = ctx.enter_context(tc.tile_pool(name="wgt", bufs=2))
        djpool = ctx.enter_context(tc.tile_pool(name="dj", bufs=2))
        acc = ctx.enter_context(tc.tile_pool(name="acc", bufs=1))

        s_cols = acc.tile([P, MB * NG * 2], F32, tag="scols")
        s_fin = acc.tile([P, MB], F32, tag="sfin")
        t_fin = acc.tile([P, MBQ], F32, tag="tfin")

        # startup order tuned for earliest full-rate PE: first token chunk,
        # then W group 0 in 512-col slivers (512 keeps full DMA bandwidth),
        # bias afterwards (the psum ring gives the first drains slack)
        hT = hpool.tile([P, KP, 2, T], FP8, tag="hT")
        nc.sync.dma_start(hT[:, :, :, 0:HC], h_r[0])
        wv0 = wpool.tile([P, KP, 2, GV], FP8, tag="w")
        for a, b in ((0, 512), (512, 1024), (1024, 2000)):
            nc.sync.dma_start(wv0[:, :, :, a:b], W_r[:, :, :, a:b])
        bb0 = bpool.tile([P, GV], F32, tag="bias")
        nc.sync.dma_start(bb0[:], bias_d[0:GV].partition_broadcast(P))
        for c in range(1, T // HC):
            nc.sync.dma_start(hT[:, :, :, c * HC:(c + 1) * HC], h_r[c])

        # prefetch group 1 and the rowdot operands behind it
        wv1 = wpool.tile([P, KP, 2, GV], FP8, tag="w")
        nc.sync.dma_start(wv1[:], W_r[:, :, :, GV:2 * GV])
        bb1 = bpool.tile([P, GV], F32, tag="bias")
        nc.sync.dma_start(bb1[:], bias_d[GV:2 * GV].partition_broadcast(P))
        rowdot_io = []
        for mb in range(MBQ):
            hg = hgpool.tile([P, D], F32, tag="hg")
            nc.sync.dma_start(hg[:], hn_d[mb * P:(mb + 1) * P, :])
            wgt = wgpool.tile([P, D], F32, tag="wgt")
            nc.sync.dma_start(wgt[:], wg_d[mb * P:(mb + 1) * P, :])
            rowdot_io.append((hg, wgt))

        wtiles = [wv0, wv1]
        btiles = [bb0, bb1]
        for g in range(NG):
            wv, bb = wtiles[g], btiles[g]
            if g + 2 < NG:          # keep the double-buffer one group ahead
                wnx = wpool.tile([P, KP, 2, GV], FP8, tag="w")
                nc.sync.dma_start(
                    wnx[:], W_r[:, :, :, (g + 2) * GV:(g + 3) * GV])
                bnx = bpool.tile([P, GV], F32, tag="bias")
                nc.sync.dma_start(
                    bnx[:], bias_d[(g + 2) * GV:(g + 3) * GV].partition_broadcast(P))
                wtiles.append(wnx)
                btiles.append(bnx)

            if g == 1:
                # exact-f32 target logits on the idle Pool engine:
                # t = sum_d hn * W[label], accumulated per token row
                for mb in range(MBQ):
                    hg, wgt = rowdot_io[mb]
                    dj = djpool.tile([P, D], F32, tag="dj")
                    nc.gpsimd.scalar_tensor_tensor(
                        dj[:], hg[:], 1.0, wgt[:],
                        op0=Alu.mult, op1=Alu.mult,
                        accum_out=t_fin[:, mb:mb + 1])

            for m in range(MB):
                lhsT = hT[:, :, :, m * P:(m + 1) * P]
                for half in range(2):
                    pt = pspool.tile([P, 2, 512], F32, tag="ps")
                    for bk in range(2):
                        c0 = half * (2 * BANK) + bk * BANK
                        for kp in range(KP):
                            nc.tensor.matmul(
                                pt[:, bk, 0:BANK], lhsT[:, kp, :, :],
                                wv[:, kp, :, c0:c0 + BANK],
                                start=(kp == 0), stop=(kp == KP - 1),
                                perf_mode=mybir.MatmulPerfMode.DoubleRow,
                            )
                    psl = pt[:, 0:2, 0:BANK]
                    bbv = bb[:, half * 2 * BANK:(half + 1) * 2 * BANK]
                    bbv = bbv.rearrange("p (b c) -> p b c", c=BANK)
                    nc.vector.scalar_tensor_tensor(
                        psl, psl, 1.0 / W_SCALE, bbv, op0=Alu.mult, op1=Alu.add)
                    et = epool.tile([P, 2, BANK], F32, tag="ejunk")
                    col = m * (NG * 2) + g * 2 + half
                    nc.scalar.activation(
                        et[:], psl, Act.Exp, accum_out=s_cols[:, col:col + 1])
                if g == NG - 1:
                    nc.vector.tensor_reduce(
                        s_fin[:, m:m + 1],
                        s_cols[:, m * (NG * 2):(m + 1) * (NG * 2)],
                        axis=mybir.AxisListType.X, op=Alu.add)

        nc.sync.dma_start(s_out[:], s_fin[:])
        nc.sync.dma_start(t_out[:], t_fin[:])

    nc.compile()
    return nc


_NC_CACHE = {}


def _get_program():
    if "nc" not in _NC_CACHE:
        _NC_CACHE["nc"] = _build()
    return _NC_CACHE["nc"]


def _to_sbuf_layout(a):
    """[D, X] f32/fp8 -> [128, KP, 2, X] matching d = kp*256 + j*128 + ki."""
    X = a.shape[1]
    return np.ascontiguousarray(
        a.reshape(KP, 2, P, X).transpose(2, 0, 1, 3))


def kernel(hidden_states, head_weight, head_bias, loss_weight, labels,
           chunk_size=None, **_unused):
    hidden = np.asarray(hidden_states, dtype=np.float32)
    W = np.asarray(head_weight, dtype=np.float32)
    bias = np.asarray(head_bias, dtype=np.float32)
    lw = np.asarray(loss_weight, dtype=np.float32)
    labels = np.asarray(labels).astype(np.int64)

    assert hidden.shape == (N_TOK, D) and W.shape == (V, D)

    nc = _get_program()

    hq = hidden.astype(FP8NP)                       # [N, D] fp8
    Wq = (W * W_SCALE).astype(FP8NP)                # [V, D] fp8, x64
    Wg = W[labels]                                  # gathered rows [N, D] f32

    in_maps = []
    for c in range(N_CORES):
        i, j = divmod(c, VOC_SPLIT)
        tok = slice(i * T, (i + 1) * T)
        voc = slice(j * VC, (j + 1) * VC)
        # rowdot tokens: quarter j of token half i
        rtok = slice(i * T + j * MBQ * P, i * T + (j + 1) * MBQ * P)
        in_maps.append(dict(
            h=_to_sbuf_layout(hq[tok].T),
            W=_to_sbuf_layout(Wq[voc].T),
            bias=np.ascontiguousarray(bias[voc]),
            hn=np.ascontiguousarray(hidden[rtok]),
            wg=np.ascontiguousarray(Wg[rtok]),
        ))
    res = run_bass_kernel_spmd(nc, in_maps, list(range(N_CORES)))

    # unshard + host-side combine (the scalar all_reduce of the hint):
    # sum the 4 vocab-quarter exp-sums per token, then logsumexp
    s = np.zeros((TOK_SPLIT, T), np.float64)
    tgt = np.zeros(N_TOK, np.float64)
    for c in range(N_CORES):
        i, j = divmod(c, VOC_SPLIT)
        r = res.results[c]
        s[i] += r["s_out"].T.reshape(-1).astype(np.float64)    # token = m*128+p
        rtok = slice(i * T + j * MBQ * P, i * T + (j + 1) * MBQ * P)
        tgt[rtok] = r["t_out"].T.reshape(-1).astype(np.float64)
    lse = np.log(s.reshape(-1))
    tgt = tgt + bias[labels].astype(np.float64)     # rowdot excludes bias
    nll = lse - tgt
    w64 = lw.astype(np.float64)
    loss = (w64 * nll).sum() / max(w64.sum(), 1.0)
    return np.float32(loss)
